# revision 6
# baseline (speedup 1.0000x reference)
"""Trainium2 Bass kernel v2 for the dense transformer block, 8 NeuronCores.

Sharding: core c handles batch b=c//2 and half hf=c%2 of that batch's 2048
tokens ("own" tokens = cols 0:1024 after a host-side roll). K/V are computed
for the full 2048-token batch on both cores of a pair (no collectives).

Design:
- No DRAM scratch roundtrip: LN gammas/betas are folded into the weights and
  biases host-side, so LN reduces to h = x*rstd_bcast + nm_bcast with two
  per-token-block broadcast matmuls.
- Activations/weights bf16 on device (f32 accumulation in PSUM); the x /
  residual stream stays f32 (f32r for matmul-fed tiles).
- Weights host-pretiled into contiguous bf16 blocks in exact load order,
  streamed exactly once, issued from the otherwise-idle Pool queue so they
  prefetch ahead of ScalarE work.
- All consts packed into 2 DMAs.
- LN stats/apply interleaved across token blocks so PE never waits on the
  cross-engine stats chain.
- Wide PSUM tiles (1024/2048 cols) so exp/copies/gelu run as few big
  ScalarE/DVE instructions.
"""

import numpy as np

from contextlib import ExitStack

import concourse.bass as bass
import concourse.bacc as bacc
import concourse.tile as tile
import concourse.mybir as mybir

F32 = mybir.dt.float32
F32R = mybir.dt.float32r
BF16 = mybir.dt.bfloat16
AF = mybir.ActivationFunctionType
OP = mybir.AluOpType

EPS = 1e-5

E = 1024
H = 16
D = 64
MLP = 4096
T_OWN = 1024
T_FULL = 2048
NE = E // 128          # 8 feature tiles
NM = MLP // 128        # 32 mlp tiles
NQB = T_OWN // 512     # 2 own-token blocks
NFB = T_FULL // 512    # 4 full-token blocks
NTK = T_FULL // 128    # 16 key tiles
NHP = H // 2           # 8 head pairs


class Cfg:
    def __init__(self, repeat=1, upto=7):
        self.repeat = repeat
        self.upto = upto


def build(cfg: Cfg):
    nc = bacc.Bacc("TRN2", target_bir_lowering=False, debug=False)

    d = {}
    d["xT"] = nc.dram_tensor("xT", [E, T_FULL], F32, kind="ExternalInput")
    # host-pretiled bf16 weight blocks, flattened [n_blocks*128, cols]
    d["wq"] = nc.dram_tensor("wq", [4 * NE * 128, 256], BF16, kind="ExternalInput")
    d["wk"] = nc.dram_tensor("wk", [8 * NE * 128, 128], BF16, kind="ExternalInput")
    d["wv"] = nc.dram_tensor("wv", [2 * NE * 128, 512], BF16, kind="ExternalInput")
    d["wfc"] = nc.dram_tensor("wfc", [4 * NE * 128, 256], BF16, kind="ExternalInput")
    d["w1"] = nc.dram_tensor("w1", [16 * NE * 128, 256], BF16, kind="ExternalInput")
    d["w2"] = nc.dram_tensor("w2", [4 * NM * 128, 256], BF16, kind="ExternalInput")
    # packed consts: columns [qb(8) kb(8) fcb(8) b2(8) b1(32) ones(1)] and a
    # row [ones(128) ones64(64) vb0(512) vb1(512) eps(1)]
    d["cpc"] = nc.dram_tensor("cpc", [128, 65], F32, kind="ExternalInput")
    d["cpr"] = nc.dram_tensor("cpr", [1, 1217], F32, kind="ExternalInput")
    d["out"] = nc.dram_tensor("out", [E, T_OWN], F32, kind="ExternalOutput")

    with tile.TileContext(nc) as tc, nc.allow_low_precision(
        reason="bf16 matmul operands by design"
    ):
        if cfg.repeat == 1:
            _body(nc, tc, d, cfg.upto)
        else:
            with tc.For_i(0, cfg.repeat, 1):
                _body(nc, tc, d, cfg.upto)
    nc.compile()
    return nc


def _ln_rows(nc, row_pool, bc_ps, s1, s2, eps_t, ones_row):
    """From PSUM sums s1=sum(x), s2=sum(x^2) over E, produce [128,512] PSUM
    broadcasts of rstd and -mu*rstd."""
    m_row = row_pool.tile([1, 512], F32, tag="mrow", name="mr", bufs=2)
    nc.vector.tensor_scalar_mul(m_row[:], s1[:], 1.0 / E)
    v_row = row_pool.tile([1, 512], F32, tag="vrow", name="vr", bufs=2)
    nc.vector.scalar_tensor_tensor(
        v_row[:], m_row[:], -1.0, m_row[:], op0=OP.mult, op1=OP.mult)
    nc.vector.scalar_tensor_tensor(
        v_row[:], s2[:], 1.0 / E, v_row[:], op0=OP.mult, op1=OP.add)
    sd_row = row_pool.tile([1, 512], F32, tag="sdrow", name="sd", bufs=2)
    nc.scalar.activation(sd_row[:], v_row[:], AF.Sqrt, bias=eps_t, scale=1.0)
    r_row = row_pool.tile([1, 512], F32R, tag="rrow", name="rr", bufs=2)
    nc.vector.reciprocal(r_row[:], sd_row[:])
    nm_row = row_pool.tile([1, 512], F32R, tag="nmrow", name="nm", bufs=2)
    nc.vector.scalar_tensor_tensor(
        nm_row[:], m_row[:], -1.0, r_row[:].bitcast(F32),
        op0=OP.mult, op1=OP.mult)
    r_b = bc_ps.tile([128, 512], F32, tag="rb")
    nc.tensor.matmul(r_b[:], ones_row, r_row[:], start=True, stop=True)
    nm_b = bc_ps.tile([128, 512], F32, tag="nmb")
    nc.tensor.matmul(nm_b[:], ones_row, nm_row[:], start=True, stop=True)
    return r_b, nm_b


def _body(nc, tc, d, upto=7):
    xT, out = d["xT"], d["out"]

    with ExitStack() as ctx:
        consts = ctx.enter_context(tc.tile_pool(name="consts", bufs=1))
        cc = consts.tile([128, 65], F32R)
        nc.sync.dma_start(cc[:], d["cpc"].ap()[:, :].bitcast(F32R))
        cr = consts.tile([1, 1217], F32R)
        nc.sync.dma_start(cr[:], d["cpr"].ap()[:, :].bitcast(F32R))

        qb_c = [cc[:, e:e + 1].bitcast(F32) for e in range(8)]
        kb_c = [cc[:, 8 + e:9 + e].bitcast(F32) for e in range(8)]
        fcb_c = [cc[:, 16 + e:17 + e].bitcast(F32) for e in range(8)]
        b2_c = [cc[:, 24 + e:25 + e].bitcast(F32) for e in range(8)]
        b1_c = [cc[:, 32 + m:33 + m].bitcast(F32) for m in range(32)]
        ones_col = cc[:, 64:65]
        ones_row = cr[0:1, 0:128]
        ones64_row = cr[0:1, 128:192]
        vb_row = [cr[0:1, 192:704], cr[0:1, 704:1216]]
        eps_t = cr[0:1, 1216:1217].bitcast(F32)

        # ---------------- resident activation arrays ----------------
        av_pool = ctx.enter_context(tc.tile_pool(name="avp", bufs=NE))
        av_tiles = [av_pool.tile([128, T_OWN], BF16, tag="avt", name="avt")
                    for _ in range(NE)]

        sA = ctx.enter_context(ExitStack())
        q_pool = sA.enter_context(tc.tile_pool(name="qp", bufs=NHP))
        q_tiles = [q_pool.tile([128, T_OWN], BF16, tag="qt", name="qt")
                   for _ in range(NHP)]
        k_pool = sA.enter_context(tc.tile_pool(name="kp", bufs=NHP))
        k_tiles = [k_pool.tile([128, T_FULL], BF16, tag="kt", name="kt")
                   for _ in range(NHP)]
        v_pool = sA.enter_context(tc.tile_pool(name="vp", bufs=NTK))
        v_tiles = [v_pool.tile([128, H, 65], BF16, tag="vt", name="vt")
                   for _ in range(NTK)]
        hb_pool = sA.enter_context(tc.tile_pool(name="hbp", bufs=NE))
        h_tiles = [hb_pool.tile([128, T_FULL], BF16, tag="ht", name="ht")
                   for _ in range(NE)]

        # ======== Phase 1: x load + LN1 stats + h (bf16), interleaved ========
        with ExitStack() as p1:
            xs_pool = p1.enter_context(tc.tile_pool(name="xsp", bufs=2 * NE + 2))
            sq_pool = p1.enter_context(tc.tile_pool(name="sqp", bufs=2))
            st_ps = p1.enter_context(tc.tile_pool(name="stps", bufs=2, space="PSUM"))
            bc_ps = p1.enter_context(tc.tile_pool(name="bcps", bufs=2, space="PSUM"))
            row_pool = p1.enter_context(tc.tile_pool(name="rows", bufs=2))
            tmp_pool = p1.enter_context(tc.tile_pool(name="tmpp", bufs=3))

            def ln1_stats(tb):
                sl = slice(tb * 512, (tb + 1) * 512)
                xts = []
                for e in range(NE):
                    t = xs_pool.tile([128, 512], F32R, tag="xs", name="xs")
                    nc.sync.dma_start(
                        t[:], xT.ap()[e * 128:(e + 1) * 128, sl].bitcast(F32R))
                    xts.append(t[:])
                s1 = st_ps.tile([1, 512], F32, tag="s1")
                s2 = st_ps.tile([1, 512], F32, tag="s2")
                for e in range(NE):
                    sq = sq_pool.tile([128, 512], F32R, tag="sq", name="sq")
                    nc.gpsimd.tensor_tensor(sq[:], xts[e].bitcast(F32),
                                            xts[e].bitcast(F32), OP.mult)
                    nc.tensor.matmul(s1[:], ones_col, xts[e],
                                     start=(e == 0), stop=(e == NE - 1))
                    nc.tensor.matmul(s2[:], ones_col, sq[:],
                                     start=(e == 0), stop=(e == NE - 1))
                return sl, xts, s1, s2

            def ln1_apply(job):
                sl, xts, s1, s2 = job
                r_b, nm_b = _ln_rows(nc, row_pool, bc_ps, s1, s2, eps_t, ones_row)
                for e in range(NE):
                    tmp = tmp_pool.tile([128, 512], F32, tag="tmp", name="tmp")
                    nc.vector.tensor_tensor(tmp[:], xts[e].bitcast(F32), r_b[:],
                                            OP.mult)
                    nc.vector.tensor_tensor(
                        h_tiles[e][:, sl], tmp[:], nm_b[:], OP.add)

            pend = None
            for tb in range(NFB):
                job = ln1_stats(tb)
                if pend is not None:
                    ln1_apply(pend)
                pend = job
            ln1_apply(pend)

        # ======== Phase 2: QKV projections ========
        if upto < 2:
            return
        with ExitStack() as p2:
            w_pool = p2.enter_context(tc.tile_pool(name="wp", bufs=4))

            # --- Q: 4 passes x 2 otiles, accumulate over e; [128,1024] psum ---
            with tc.tile_pool(name="qps", bufs=4, space="PSUM") as q_ps:
                for p in range(4):
                    ps = [q_ps.tile([128, T_OWN], F32, tag="qacc", name="qacc")
                          for _ in range(2)]
                    wt = w_pool.tile([128, NE * 256], BF16, tag="wq", name="wq",
                                     bufs=2)
                    nc.sync.dma_start(
                        wt[:].rearrange("p (b c) -> p b c", c=256),
                        d["wq"].ap()[p * NE * 128:(p + 1) * NE * 128, :]
                        .rearrange("(b p) c -> p b c", p=128))
                    for e in range(NE):
                        for j in range(2):
                            for tqb in range(NQB):
                                nc.tensor.matmul(
                                    ps[j][:, tqb * 512:(tqb + 1) * 512],
                                    wt[:, e * 256 + j * 128:e * 256 + (j + 1) * 128],
                                    h_tiles[e][:, tqb * 512:(tqb + 1) * 512],
                                    start=(e == 0), stop=(e == NE - 1))
                    for j in range(2):
                        o = p * 2 + j  # feature tile o == head-pair o
                        nc.scalar.activation(q_tiles[o][:, :], ps[j][:],
                                             AF.Identity, bias=qb_c[o], scale=1.0)

            # --- K: 8 passes x 1 ktile x 4 tb; [128,2048] psum ---
            with tc.tile_pool(name="kps", bufs=2, space="PSUM") as k_ps:
                for p in range(NE):
                    ps = k_ps.tile([128, T_FULL], F32, tag="kacc", name="kacc")
                    wt = w_pool.tile([128, NE * 128], BF16, tag="wk", name="wk",
                                     bufs=2)
                    nc.sync.dma_start(
                        wt[:].rearrange("p (b c) -> p b c", c=128),
                        d["wk"].ap()[p * NE * 128:(p + 1) * NE * 128, :]
                        .rearrange("(b p) c -> p b c", p=128))
                    for e in range(NE):
                        for tb in range(NFB):
                            nc.tensor.matmul(
                                ps[:, tb * 512:(tb + 1) * 512],
                                wt[:, e * 128:(e + 1) * 128],
                                h_tiles[e][:, tb * 512:(tb + 1) * 512],
                                start=(e == 0), stop=(e == NE - 1))
                    nc.scalar.activation(k_tiles[p][:, :], ps[:],
                                         AF.Identity, bias=kb_c[p], scale=1.0)

            # --- V: tokens-on-partitions; both halves in one [128,1024] psum ---
            with tc.tile_pool(name="vps", bufs=3, space="PSUM") as v_ps, \
                 tc.tile_pool(name="vbps", bufs=1, space="PSUM") as vb_ps:
                vbp = vb_ps.tile([128, 1024], F32, tag="vb")
                for vh in range(2):
                    nc.tensor.matmul(vbp[:, vh * 512:(vh + 1) * 512], ones_row,
                                     vb_row[vh], start=True, stop=True)
                vbias_b = w_pool.tile([128, 1024], F32, tag="vbb", name="vbb",
                                      bufs=1)
                nc.vector.tensor_copy(vbias_b[:], vbp[:])
                wv_res = []
                for vh in range(2):
                    wt = w_pool.tile([128, NE * 512], BF16, tag=f"wv{vh}",
                                     name="wv", bufs=1)
                    nc.sync.dma_start(
                        wt[:].rearrange("p (b c) -> p b c", c=512),
                        d["wv"].ap()[vh * NE * 128:(vh + 1) * NE * 128, :]
                        .rearrange("(b p) c -> p b c", p=128))
                    wv_res.append(wt)
                for tk in range(NTK):
                    nc.vector.memset(v_tiles[tk][:, :, 64:65], 1.0)
                for tk in range(NTK):
                    ps_v = v_ps.tile([128, 1024], F32, tag="vacc", name="vacc")
                    tsl = slice(tk * 128, (tk + 1) * 128)
                    for e in range(NE):
                        for vh in range(2):
                            nc.tensor.matmul(
                                ps_v[:, vh * 512:(vh + 1) * 512],
                                h_tiles[e][:, tsl],
                                wv_res[vh][:, e * 512:(e + 1) * 512],
                                start=(e == 0), stop=(e == NE - 1))
                    nc.vector.tensor_tensor(
                        v_tiles[tk][:, :, 0:64],
                        ps_v[:].rearrange("p (h d) -> p h d", d=64),
                        vbias_b[:].rearrange("p (h d) -> p h d", d=64),
                        OP.add)

        # ======== Phase 3: attention ========
        if upto < 3:
            return
        with ExitStack() as p3:
            sc_ps = p3.enter_context(tc.tile_pool(name="scps", bufs=2, space="PSUM"))
            av_ps = p3.enter_context(tc.tile_pool(name="avps", bufs=4, space="PSUM"))
            ex_pool = p3.enter_context(tc.tile_pool(name="exp", bufs=4))
            rec_pool = p3.enter_context(tc.tile_pool(name="recp", bufs=4))

            def emit_rm(job):
                jhp, jqsl, java, javb = job
                for head, av_t in ((0, java), (1, javb)):
                    rrow = rec_pool.tile([1, 512], F32R, tag="rr", name="rr")
                    nc.vector.reciprocal(rrow[:], av_t[64:65, :])
                    rmt = sc_ps.tile([128, 1024], F32, tag="sc", name="sc")
                    rmp = rmt[0:64, 0:512]
                    nc.tensor.matmul(rmp, ones64_row, rrow[:],
                                     start=True, stop=True)
                    rms = rec_pool.tile([64, 512], F32, tag="rms", name="rms")
                    nc.vector.tensor_copy(rms[:], rmp)
                    nc.vector.tensor_tensor(
                        av_tiles[jhp][head * 64:(head + 1) * 64, jqsl],
                        av_t[0:64, :], rms[:], OP.mult)

            prev = None
            for hp in range(NHP):
                for tqb in range(NQB):
                    qsl = slice(tqb * 512, (tqb + 1) * 512)
                    ava = av_ps.tile([65, 512], F32, tag="av", name="av")
                    avb = av_ps.tile([65, 512], F32, tag="av", name="av")
                    exs = [None] * NTK
                    LAG = 2  # av of k-tile N-2 issues after scores of k-tile N,
                             # so the wide exp never stalls the PE
                    for tk in range(NTK + LAG):
                        if tk < NTK:
                            ksl = slice(tk * 128, (tk + 1) * 128)
                            # both heads' scores in one [128,1024] psum tile
                            sc = sc_ps.tile([128, 1024], F32, tag="sc", name="sc")
                            nc.tensor.matmul(sc[:, 0:512], k_tiles[hp][0:64, ksl],
                                             q_tiles[hp][0:64, qsl],
                                             start=True, stop=True)
                            nc.tensor.matmul(sc[:, 512:1024],
                                             k_tiles[hp][64:128, ksl],
                                             q_tiles[hp][64:128, qsl],
                                             start=True, stop=True)
                            ex = ex_pool.tile([128, 1024], BF16, tag="ex",
                                              name="ex")
                            nc.scalar.activation(ex[:], sc[:], AF.Exp)
                            exs[tk] = ex
                        if tk == 3 and prev is not None:
                            # normalize the PREVIOUS pair here: its reciprocal
                            # latency hides under this pair's score matmuls
                            emit_rm(prev)
                            prev = None
                        if tk >= LAG:
                            pv = tk - LAG
                            nc.tensor.matmul(ava[:], v_tiles[pv][:, 2 * hp, :],
                                             exs[pv][:, 0:512],
                                             start=(pv == 0),
                                             stop=(pv == NTK - 1))
                            nc.tensor.matmul(avb[:], v_tiles[pv][:, 2 * hp + 1, :],
                                             exs[pv][:, 512:1024],
                                             start=(pv == 0),
                                             stop=(pv == NTK - 1))
                    prev = (hp, qsl, ava, avb)
            emit_rm(prev)

        if upto < 3.7:
            return
        sA.close()  # isolate: upto=3.7 runs close but not fc
        if upto < 4:
            return  # free q/k/v/h (113KB/partition) before the MLP-side arrays

        # ======== Phase 4: fc_out + residual, with LN2 stats inlined ====
        x2_pool = ctx.enter_context(tc.tile_pool(name="x2p", bufs=NE))
        x2_tiles = [x2_pool.tile([128, T_OWN], F32R, tag="x2t", name="x2t")
                    for _ in range(NE)]
        h2_pool = ctx.enter_context(tc.tile_pool(name="h2p", bufs=NE))
        h2_tiles = [h2_pool.tile([128, T_OWN], BF16, tag="h2t", name="h2t")
                    for _ in range(NE)]
        g_pool = ctx.enter_context(tc.tile_pool(name="gp", bufs=NM))
        g_tiles = [g_pool.tile([128, T_OWN], BF16, tag="gt", name="gt")
                   for _ in range(NM)]
        with ExitStack() as p45:
            st_ps = p45.enter_context(
                tc.tile_pool(name="st2ps", bufs=2, space="PSUM"))
            sq_pool = p45.enter_context(tc.tile_pool(name="sq2p", bufs=2))
            row_pool = p45.enter_context(tc.tile_pool(name="rows2", bufs=2))
            tmp_pool = p45.enter_context(tc.tile_pool(name="tmp2p", bufs=3))
            s1t = [st_ps.tile([1, 512], F32, tag="s1", name="s1")
                   for _ in range(NQB)]
            s2t = [st_ps.tile([1, 512], F32, tag="s2", name="s2")
                   for _ in range(NQB)]
            with ExitStack() as p4:
                wf_pool = p4.enter_context(tc.tile_pool(name="wfp", bufs=4))
                xr_pool = p4.enter_context(tc.tile_pool(name="xrp", bufs=4))
                fc_ps = p4.enter_context(
                    tc.tile_pool(name="fcps", bufs=2, space="PSUM"))
                for p in range(4):
                    ps = [fc_ps.tile([128, T_OWN], F32, tag="fc", name="fc")
                          for _ in range(2)]
                    wt = wf_pool.tile([128, NE * 256], BF16, tag="wf", name="wf",
                                      bufs=2)
                    nc.sync.dma_start(
                        wt[:].rearrange("p (b c) -> p b c", c=256),
                        d["wfc"].ap()[p * NE * 128:(p + 1) * NE * 128, :]
                        .rearrange("(b p) c -> p b c", p=128))
                    for e in range(NE):
                        for j in range(2):
                            for tqb in range(NQB):
                                nc.tensor.matmul(
                                    ps[j][:, tqb * 512:(tqb + 1) * 512],
                                    wt[:, e * 256 + j * 128:e * 256 + (j + 1) * 128],
                                    av_tiles[e][:, tqb * 512:(tqb + 1) * 512],
                                    start=(e == 0), stop=(e == NE - 1))
                    for j in range(2):
                        o = p * 2 + j
                        xr = xr_pool.tile([128, 1024], F32, tag="xr", name="xr")
                        nc.sync.dma_start(xr[:],
                                          xT.ap()[o * 128:(o + 1) * 128, 0:T_OWN])
                        nc.vector.scalar_tensor_tensor(
                            x2_tiles[o][:, :], ps[j][:], fcb_c[o], xr[:],
                            op0=OP.add, op1=OP.add)
                        # LN2 stats contribution of this x2 tile (both blocks)
                        for tb in range(NQB):
                            sl = slice(tb * 512, (tb + 1) * 512)
                            sq = sq_pool.tile([128, 512], F32R, tag="sq",
                                              name="sq")
                            nc.gpsimd.tensor_tensor(
                                sq[:], x2_tiles[o][:, sl].bitcast(F32),
                                x2_tiles[o][:, sl].bitcast(F32), OP.mult)
                            nc.tensor.matmul(s1t[tb][:], ones_col,
                                             x2_tiles[o][:, sl],
                                             start=(o == 0), stop=(o == NE - 1))
                            nc.tensor.matmul(s2t[tb][:], ones_col, sq[:],
                                             start=(o == 0), stop=(o == NE - 1))

            # ======== Phase 5: LN2 rows + h2 (bf16) ========
            if upto < 5:
                return
            with ExitStack() as p5:
                bc_ps = p5.enter_context(
                    tc.tile_pool(name="bc2ps", bufs=2, space="PSUM"))
                # both rows chains first, then e-interleaved h2 writes so
                # h2[0] (which gates MLP1's first accumulation) lands early
                bcs = [_ln_rows(nc, row_pool, bc_ps, s1t[tb], s2t[tb],
                                eps_t, ones_row) for tb in range(NQB)]
                for e in range(NE):
                    for tb in range(NQB):
                        sl = slice(tb * 512, (tb + 1) * 512)
                        r_b, nm_b = bcs[tb]
                        tmp = tmp_pool.tile([128, 512], F32, tag="tmp",
                                            name="tmp")
                        nc.vector.tensor_tensor(
                            tmp[:], x2_tiles[e][:, sl].bitcast(F32), r_b[:],
                            OP.mult)
                        nc.vector.tensor_tensor(
                            h2_tiles[e][:, sl], tmp[:], nm_b[:], OP.add)

        # ======== Phase 6: MLP ========
        if upto < 6:
            return
        with ExitStack() as p6:
            w1_pool = p6.enter_context(tc.tile_pool(name="w1p", bufs=4))
            m1_ps = p6.enter_context(tc.tile_pool(name="m1ps", bufs=4, space="PSUM"))
            # 16 groups x 2 mtiles; [128,1024] psum per mtile; ring of 4 = 2 grps
            for grp in range(16):
                ps = [m1_ps.tile([128, T_OWN], F32, tag="m1", name="m1")
                      for _ in range(2)]
                wt = w1_pool.tile([128, NE * 256], BF16, tag="w1", name="w1",
                                  bufs=2)
                nc.sync.dma_start(
                    wt[:].rearrange("p (b c) -> p b c", c=256),
                    d["w1"].ap()[grp * NE * 128:(grp + 1) * NE * 128, :]
                    .rearrange("(b p) c -> p b c", p=128))
                for e in range(NE):
                    for j in range(2):
                        for tqb in range(NQB):
                            nc.tensor.matmul(
                                ps[j][:, tqb * 512:(tqb + 1) * 512],
                                wt[:, e * 256 + j * 128:e * 256 + (j + 1) * 128],
                                h2_tiles[e][:, tqb * 512:(tqb + 1) * 512],
                                start=(e == 0), stop=(e == NE - 1))
                for j in range(2):
                    m = grp * 2 + j
                    nc.scalar.activation(g_tiles[m][:, :], ps[j][:],
                                         AF.Gelu, bias=b1_c[m], scale=1.0)

        if upto < 6.3:
            return
        with ExitStack() as p7:
            w2_pool = p7.enter_context(tc.tile_pool(name="w2p", bufs=4))
            m2_ps = p7.enter_context(tc.tile_pool(name="m2ps", bufs=4, space="PSUM"))
            out_pool = p7.enter_context(tc.tile_pool(name="outp", bufs=4))
            # 4 passes x 2 otiles, accumulate over all m=32
            for p in range(4):
                ps = [m2_ps.tile([128, T_OWN], F32, tag="m2", name="m2")
                      for _ in range(2)]
                whs = []
                for h2i in range(2):
                    wh = w2_pool.tile([128, 16 * 256], BF16, tag="w2", name="w2",
                                      bufs=3)
                    r0 = (p * NM + h2i * 16) * 128
                    nc.sync.dma_start(
                        wh[:].rearrange("p (b c) -> p b c", c=256),
                        d["w2"].ap()[r0:r0 + 16 * 128, :]
                        .rearrange("(b p) c -> p b c", p=128))
                    whs.append(wh)
                for m in range(NM):
                    wt = whs[m // 16]
                    mi = m % 16
                    for j in range(2):
                        for tqb in range(NQB):
                            nc.tensor.matmul(
                                ps[j][:, tqb * 512:(tqb + 1) * 512],
                                wt[:, mi * 256 + j * 128:mi * 256 + (j + 1) * 128],
                                g_tiles[m][:, tqb * 512:(tqb + 1) * 512],
                                start=(m == 0), stop=(m == NM - 1))
                for j in range(2):
                    o = p * 2 + j
                    if upto < 6.6:
                        continue
                    ot = out_pool.tile([128, 1024], F32, tag="ot", name="ot")
                    nc.vector.scalar_tensor_tensor(
                        ot[:], ps[j][:], b2_c[o],
                        x2_tiles[o][:, :].bitcast(F32), op0=OP.add, op1=OP.add)
                    if upto < 7:
                        continue
                    nc.sync.dma_start(out.ap()[o * 128:(o + 1) * 128, :], ot[:])


# ----------------------------------------------------------------------------
# host driver
# ----------------------------------------------------------------------------
B, S = 4, 2048
_cache = {}


def _get_nc():
    if "nc" not in _cache:
        _cache["nc"] = build(Cfg())
    return _cache["nc"]


def _prep_weights(qkv_w, fc_w, fc_b, ln1_g, ln1_b, ln2_g, ln2_b, w1, b1, w2, b2):
    """Shared (core-independent) weight pretiling -> dict of device arrays."""
    scale = D ** -0.5
    wq_s = qkv_w.astype(np.float64).copy()
    wq_s[0:E] *= scale
    qkvb = (wq_s @ ln1_b.astype(np.float64)).astype(np.float32)
    WpT = (wq_s * ln1_g.astype(np.float64)[None, :]).T.astype(np.float32)  # [E, 3E]

    def blocks(mat, specs):
        return np.concatenate(
            [np.ascontiguousarray(mat[rs, cs]) for rs, cs in specs], axis=0)

    wq_blk = blocks(WpT, [
        (slice(e * 128, (e + 1) * 128), slice(p * 256, (p + 1) * 256))
        for p in range(4) for e in range(NE)])
    wk_blk = blocks(WpT, [
        (slice(e * 128, (e + 1) * 128), slice(E + p * 128, E + (p + 1) * 128))
        for p in range(NE) for e in range(NE)])
    wv_blk = blocks(WpT, [
        (slice(e * 128, (e + 1) * 128), slice(2 * E + vh * 512, 2 * E + (vh + 1) * 512))
        for vh in range(2) for e in range(NE)])

    fcT = np.ascontiguousarray(fc_w.T).astype(np.float32)  # [E, E]
    wfc_blk = blocks(fcT, [
        (slice(e * 128, (e + 1) * 128), slice(p * 256, (p + 1) * 256))
        for p in range(4) for e in range(NE)])

    w1T = (w1.astype(np.float64) * ln2_g.astype(np.float64)[None, :]).T.astype(
        np.float32)  # [E, MLP]
    b1e = (b1.astype(np.float64)
           + w1.astype(np.float64) @ ln2_b.astype(np.float64)).astype(np.float32)
    w1_blk = blocks(w1T, [
        (slice(e * 128, (e + 1) * 128), slice(g * 256, (g + 1) * 256))
        for g in range(16) for e in range(NE)])

    w2T = np.ascontiguousarray(w2.T).astype(np.float32)  # [MLP, E]
    w2_blk = blocks(w2T, [
        (slice(m * 128, (m + 1) * 128), slice(p * 256, (p + 1) * 256))
        for p in range(4) for m in range(NM)])

    cpc = np.empty((128, 65), np.float32)
    cpc[:, 0:8] = qkvb[0:E].reshape(8, 128).T
    cpc[:, 8:16] = qkvb[E:2 * E].reshape(8, 128).T
    cpc[:, 16:24] = fc_b.astype(np.float32).reshape(8, 128).T
    cpc[:, 24:32] = b2.astype(np.float32).reshape(8, 128).T
    cpc[:, 32:64] = b1e.reshape(32, 128).T
    cpc[:, 64] = 1.0
    cpr = np.empty((1, 1217), np.float32)
    cpr[0, 0:192] = 1.0
    cpr[0, 192:704] = qkvb[2 * E:2 * E + 512]
    cpr[0, 704:1216] = qkvb[2 * E + 512:3 * E]
    cpr[0, 1216] = EPS

    def bf16(a):
        import jax.numpy as jnp
        return np.asarray(jnp.asarray(a, dtype=jnp.bfloat16))

    return {
        "wq": bf16(wq_blk), "wk": bf16(wk_blk), "wv": bf16(wv_blk),
        "wfc": bf16(wfc_blk), "w1": bf16(w1_blk), "w2": bf16(w2_blk),
        "cpc": cpc, "cpr": cpr,
    }


def _host_prepare(x_b, roll, shared):
    xr = np.roll(x_b, -roll, axis=0)
    m = dict(shared)
    m["xT"] = np.ascontiguousarray(xr.T)
    return m


def kernel(x, qkv_w, fc_w, fc_b, ln1_g, ln1_b, ln2_g, ln2_b, w1, b1, w2, b2):
    from concourse.bass_utils import run_bass_kernel_spmd

    x = np.ascontiguousarray(np.asarray(x, dtype=np.float32))
    shared = _prep_weights(
        np.asarray(qkv_w, np.float32), np.asarray(fc_w, np.float32),
        np.asarray(fc_b, np.float32), np.asarray(ln1_g, np.float32),
        np.asarray(ln1_b, np.float32), np.asarray(ln2_g, np.float32),
        np.asarray(ln2_b, np.float32), np.asarray(w1, np.float32),
        np.asarray(b1, np.float32), np.asarray(w2, np.float32),
        np.asarray(b2, np.float32))
    nc = _get_nc()
    in_maps = []
    for c in range(8):
        b, hf = c // 2, c % 2
        in_maps.append(_host_prepare(x[b], hf * (S // 2), shared))
    res = run_bass_kernel_spmd(nc, in_maps, list(range(8)))
    out = np.empty((B, S, E), np.float32)
    for c in range(8):
        b, hf = c // 2, c % 2
        out[b, hf * (S // 2):(hf + 1) * (S // 2), :] = res.results[c]["out"].T
    return out
